# revision 3
# baseline (speedup 1.0000x reference)
"""Multi-head causal attention (B=4, S=2048, D=1024, H=16, hd=64) on 8 TRN2 cores.

Sharding: core c -> (batch b = c//2, head-group hg = c%2 of 8 heads).
Each core computes its batch's QKV projection for its 8 heads (tensor-parallel
column split of Wq/Wk/Wv), causal attention, and a partial output projection
(row-parallel split of Wo). Host sums the two partials per batch and adds bias.

Device-side layout avoids all transposes:
  - host passes x[b] transposed (xT [1024, 2048])
  - Q^T/K^T computed as [d, seq] via lhsT=W tile, rhs=xT
  - V computed natural [seq, d] via lhsT=xT tile, rhs=W, stored with a ones
    column per head (v_aug, M=65) so the PV matmul also accumulates the
    softmax denominator
  - scores computed as S^T [keys, q]; exp on ACT (scale=1/8); causal masking
    by skipping fully-masked key tiles + DVE multiply with an affine_select
    mask on diagonal tiles
  - 1/denom broadcast across partitions via a K=1 matmul, normalize on DVE
    writing straight into ctxT [feat, q] which is the lhsT of the out-proj
All matmuls in float32r (full PE rate at N=512).
"""
import os
import sys

import numpy as np

try:
    import concourse  # noqa: F401
except ImportError:
    sys.path.insert(0, "/opt/trn_rl_repo")

import concourse.bass as bass  # noqa: F401  (bass must import before bacc)
import concourse.mybir as mybir
import concourse.tile as tile
from concourse import bacc
from concourse.bass_utils import run_bass_kernel_spmd

F32 = mybir.dt.float32
F32R = mybir.dt.float32r
EXP = mybir.ActivationFunctionType.Exp

B, S, DM = 4, 2048, 1024          # batch, seq, model dim
H, HD = 16, 64                    # total heads, head dim
HG = 8                            # heads per core (head group)
DG = HG * HD                      # 512 = feature dim per core
N = 512                           # matmul moving free dim
P = 128                           # partitions
NQT = S // N                      # 4 q-tiles of 512
NKT = S // P                      # 16 key tiles of 128
NMT = DM // P                     # 8 model-dim tiles

LOOKAHEAD = 2                     # score-matmul lookahead before PV matmuls

_cached = {}


def _build():
    nc = bacc.Bacc("TRN2", target_bir_lowering=False, debug=False)

    xT_d = nc.dram_tensor("xT", [DM, S], F32, kind="ExternalInput").ap()
    wq_d = nc.dram_tensor("wq", [DM, DG], F32, kind="ExternalInput").ap()
    wk_d = nc.dram_tensor("wk", [DM, DG], F32, kind="ExternalInput").ap()
    wv_d = nc.dram_tensor("wv", [DM, DG], F32, kind="ExternalInput").ap()
    wo_d = nc.dram_tensor("wo", [DG, DM], F32, kind="ExternalInput").ap()
    out_d = nc.dram_tensor("out", [S, DM], F32, kind="ExternalOutput").ap()

    with tile.TileContext(nc) as tc:
        with (
            nc.allow_low_precision(reason="fp32r matmul staging"),
            tc.tile_pool(name="persist", bufs=1) as persist,
        ):
            # ---- persistent tiles ----
            qT = [persist.tile([P, S], F32R, name=f"qT{i}") for i in range(4)]
            kT = [persist.tile([P, S], F32R, name=f"kT{i}") for i in range(4)]
            # v_aug[si]: [128, 8 heads, 65] (64 V cols + ones col per head)
            v_aug = [persist.tile([P, HG, HD + 1], F32R, name=f"vaug{i}")
                     for i in range(NKT)]
            # wide causal mask: wide[k, u] = 1 iff k + 384 <= u; mask for
            # diagonal offset j (=128*j) is wide[:, 384-128j : 896-128j]
            maskw = persist.tile([P, 896], F32R, name="maskw")
            ones64 = persist.tile([1, HD], F32R, name="ones64")

            # ---- phase 1: load + QKV projections ----
            with (
                tc.tile_pool(name="p1", bufs=1) as p1,
                tc.tile_pool(name="p1ps", bufs=4, space="PSUM") as p1ps,
            ):
                mask_f32 = p1.tile([P, 896], F32)
                nc.gpsimd.memset(mask_f32[:], 1.0)
                nc.gpsimd.affine_select(
                    out=maskw[:], in_=mask_f32[:],
                    compare_op=mybir.AluOpType.is_ge,
                    fill=0.0, base=-384,
                    pattern=[[1, 896]], channel_multiplier=-1,
                )  # keep where u - k - 384 >= 0
                ones_f32 = p1.tile([P, HD], F32)
                nc.gpsimd.memset(ones_f32[:], 1.0)
                nc.any.tensor_copy(ones64[:], ones_f32[:1, :])

                xTt = []
                for mi in range(NMT):
                    xt = p1.tile([P, S], F32R, name=f"xT{mi}")
                    nc.gpsimd.dma_start(out=xt[:], in_=xT_d[mi * P:(mi + 1) * P, :])
                    xTt.append(xt)

                def proj_T(w_d, dst):
                    # dst[dt][:, qi*N:+N] = (W^T x^T) tile; lhsT=W, rhs=xT
                    wt = []
                    for mi in range(NMT):
                        w = p1.tile([P, DG], F32R, tag=f"w{mi}", name=f"w{mi}")
                        nc.gpsimd.dma_start(out=w[:], in_=w_d[mi * P:(mi + 1) * P, :])
                        wt.append(w)
                    for dt in range(4):
                        for qi in range(NQT):
                            ps = p1ps.tile([P, N], F32, tag="qkv", name="ps")
                            for mi in range(NMT):
                                nc.tensor.matmul(
                                    ps[:], wt[mi][:, dt * P:(dt + 1) * P],
                                    xTt[mi][:, qi * N:(qi + 1) * N],
                                    start=(mi == 0), stop=(mi == NMT - 1),
                                )
                            nc.any.tensor_copy(dst[dt][:, qi * N:(qi + 1) * N], ps[:])

                proj_T(wq_d, qT)
                proj_T(wk_d, kT)

                # V natural layout: lhsT = xT tile, rhs = Wv
                wvt = []
                for mi in range(NMT):
                    w = p1.tile([P, DG], F32R, tag=f"w{mi}", name=f"wv{mi}")
                    nc.gpsimd.dma_start(out=w[:], in_=wv_d[mi * P:(mi + 1) * P, :])
                    wvt.append(w)
                for si in range(NKT):
                    ps = p1ps.tile([P, N], F32, tag="qkv", name="psv")
                    for mi in range(NMT):
                        nc.tensor.matmul(
                            ps[:], xTt[mi][:, si * P:(si + 1) * P], wvt[mi][:],
                            start=(mi == 0), stop=(mi == NMT - 1),
                        )
                    nc.any.tensor_copy(
                        v_aug[si][:, :, :HD],
                        ps.rearrange("p (h d) -> p h d", d=HD),
                    )
                    nc.any.tensor_copy(
                        v_aug[si][:, :, HD],
                        ones_f32[:, :HG],
                    )

            # ---- phases 2+3: attention + out-proj ----
            with (
                tc.tile_pool(name="p2", bufs=1) as p2,
                tc.tile_pool(name="scps", bufs=3, space="PSUM") as scps,
                tc.tile_pool(name="ctxps", bufs=2, space="PSUM") as ctxps,
                tc.tile_pool(name="bcps", bufs=1, space="PSUM") as bcps,
                tc.tile_pool(name="oups", bufs=2, space="PSUM") as oups,
            ):
                wot = []
                for ft in range(4):
                    w = p2.tile([P, DM], F32R, name=f"wo{ft}")
                    nc.gpsimd.dma_start(out=w[:], in_=wo_d[ft * P:(ft + 1) * P, :])
                    wot.append(w)
                ctxT = [p2.tile([P, S], F32R, name=f"ctxT{i}") for i in range(4)]

                for qi in range(NQT):
                    for h in range(HG):
                        t, hb = h // 2, (h % 2) * HD
                        nk = 4 * qi + 4
                        ctx_acc = ctxps.tile([HD + 1, N], F32, tag="ctx", name="ctx_acc")

                        def scores(ki):
                            sc = scps.tile([P, N], F32, tag="sc", name="sc")
                            nc.tensor.matmul(
                                sc[:],
                                kT[t][hb:hb + HD, ki * P:(ki + 1) * P],
                                qT[t][hb:hb + HD, qi * N:(qi + 1) * N],
                                start=True, stop=True,
                            )
                            ex = p2.tile([P, N], F32R, tag="ex", bufs=LOOKAHEAD + 2,
                                         name="ex")
                            nc.scalar.activation(ex[:], sc[:], EXP, scale=0.125)
                            off = ki - 4 * qi
                            if off >= 0:
                                m0 = 384 - 128 * off
                                nc.vector.tensor_mul(
                                    ex[:], ex[:], maskw[:, m0:m0 + N])
                            return ex

                        def pv(ki, ex):
                            nc.tensor.matmul(
                                ctx_acc[:], v_aug[ki][:, h, :], ex[:],
                                start=(ki == 0), stop=(ki == nk - 1),
                            )

                        exs = []
                        for ki in range(nk):
                            exs.append(scores(ki))
                            if ki >= LOOKAHEAD:
                                pv(ki - LOOKAHEAD, exs[ki - LOOKAHEAD])
                                exs[ki - LOOKAHEAD] = None
                        for ki in range(max(0, nk - LOOKAHEAD), nk):
                            pv(ki, exs[ki])

                        recip = p2.tile([1, N], F32R, tag="recip", bufs=2, name="recip")
                        nc.vector.reciprocal(recip[:], ctx_acc[HD:HD + 1, :])
                        bc = bcps.tile([HD, N], F32, tag="bc", name="bc")
                        nc.tensor.matmul(bc[:], ones64[:], recip[:],
                                         start=True, stop=True)
                        bc_sb = p2.tile([HD, N], F32R, tag="bcsb", bufs=2, name="bcsb")
                        nc.any.tensor_copy(bc_sb[:], bc[:])
                        nc.vector.tensor_mul(
                            ctxT[t][hb:hb + HD, qi * N:(qi + 1) * N],
                            ctx_acc[:HD, :], bc_sb[:],
                        )

                    # out-proj for this q-tile (4 sub-tiles of 128 queries)
                    for s4 in range(4):
                        s = qi * 4 + s4
                        ost = p2.tile([P, DM], F32, tag="ost", bufs=2, name="ost")
                        for nt in range(2):
                            ps = oups.tile([P, N], F32, tag="ou", name="ou")
                            for ft in range(4):
                                nc.tensor.matmul(
                                    ps[:],
                                    ctxT[ft][:, s * P:(s + 1) * P],
                                    wot[ft][:, nt * N:(nt + 1) * N],
                                    start=(ft == 0), stop=(ft == 3),
                                )
                            nc.any.tensor_copy(ost[:, nt * N:(nt + 1) * N], ps[:])
                        nc.sync.dma_start(
                            out=out_d[s * P:(s + 1) * P, :], in_=ost[:])

    nc.compile()
    return nc


def _get_nc():
    if "nc" not in _cached:
        _cached["nc"] = _build()
    return _cached["nc"]


def kernel(x, Wq, Wk, Wv, Wo, bo):
    x = np.asarray(x, dtype=np.float32)
    Wq = np.asarray(Wq, dtype=np.float32)
    Wk = np.asarray(Wk, dtype=np.float32)
    Wv = np.asarray(Wv, dtype=np.float32)
    Wo = np.asarray(Wo, dtype=np.float32)
    bo = np.asarray(bo, dtype=np.float32)

    nc = _get_nc()
    in_maps = []
    for c in range(8):
        b, hg = c // 2, c % 2
        cs = slice(hg * DG, (hg + 1) * DG)
        in_maps.append({
            "xT": np.ascontiguousarray(x[b].T),
            "wq": np.ascontiguousarray(Wq[:, cs]),
            "wk": np.ascontiguousarray(Wk[:, cs]),
            "wv": np.ascontiguousarray(Wv[:, cs]),
            "wo": np.ascontiguousarray(Wo[cs, :]),
        })
    res = run_bass_kernel_spmd(nc, in_maps, list(range(8)))
    out = np.empty((B, S, DM), np.float32)
    for b in range(B):
        out[b] = res.results[2 * b]["out"] + res.results[2 * b + 1]["out"] + bo
    return out


if __name__ == "__main__":
    rng = np.random.default_rng(0)
    ins = {
        "x": rng.standard_normal((B, S, DM), dtype=np.float32),
        "Wq": rng.standard_normal((DM, DM), dtype=np.float32) / 32,
        "Wk": rng.standard_normal((DM, DM), dtype=np.float32) / 32,
        "Wv": rng.standard_normal((DM, DM), dtype=np.float32) / 32,
        "Wo": rng.standard_normal((DM, DM), dtype=np.float32) / 32,
        "bo": rng.standard_normal((DM,), dtype=np.float32) * 0.01,
    }
    out = kernel(**ins)
    print("kernel ran, out shape", out.shape, "mean", float(np.abs(out).mean()))
